# revision 19
# baseline (speedup 1.0000x reference)
"""Multi-head attention Trainium2 kernel (B=2, S=2048, D=1024, H=16).

Sharding: 8 cores = 2 batches x 4 head-groups (4 heads each).
Each core computes its head-group's attention for its batch plus the
partial output projection (Wo columns for its heads); the host sums the
4 partials per batch.

Device layout is feature-on-partition throughout (inputs pre-transposed
and pre-cast to bf16 on host), zero on-device transposes:
  QhT/KhT = W @ x^T            [dk, s]
  S^T     = KhT^T-slices @ QhT [s_k, s_q]   (K=64, head pairs row-packed)
  P^T     = exp(S^T / 8)                    (ScalarE, fused scale)
  O^T     = V_aug^T-slices @ P^T [dk+1, s_q] (ones col => row 64 = denom)
  out^T   = Wo_g^T @ OcatT^norm  [e, s]
Softmax normalization: fp16 reciprocal of the denom row + K=1
ones-matmul broadcast across partitions + DVE multiply.
bf16 matmul operands, fp32 PSUM. bq/bk applied on device per-partition;
bv/bo folded in on host (softmax rows sum to 1 => V-bias is additive).
The boolean mask input is all-ones per the problem spec and is ignored.
"""
import sys

sys.path.insert(0, "/opt/trn_rl_repo")

import numpy as np
from contextlib import ExitStack

B, S, D, H = 2, 2048, 1024, 16
DK = D // H          # 64
G = 4                # head-groups (cores per batch)
HG = H // G          # 4 heads per group
CG = HG * DK         # 256 features per group
N_CORES = 8

_nc_cache = None
_last_in_maps = None


def _build():
    import concourse.tile as tile
    from concourse import bacc, mybir

    F32 = mybir.dt.float32
    F16 = mybir.dt.float16
    BF16 = mybir.dt.bfloat16
    Exp = mybir.ActivationFunctionType.Exp

    nc = bacc.Bacc("TRN2", target_bir_lowering=False, debug=False)

    xqT = nc.declare_dram_parameter("xqT", [D, S], BF16, isOutput=False)
    xkT = nc.declare_dram_parameter("xkT", [D, S], BF16, isOutput=False)
    xvT = nc.declare_dram_parameter("xvT", [D, S], BF16, isOutput=False)
    wqT = nc.declare_dram_parameter("wqT", [D, CG], BF16, isOutput=False)
    wkT = nc.declare_dram_parameter("wkT", [D, CG], BF16, isOutput=False)
    wvT = nc.declare_dram_parameter("wvT", [D, CG], BF16, isOutput=False)
    woT = nc.declare_dram_parameter("woT", [CG, D], BF16, isOutput=False)
    bq = nc.declare_dram_parameter("bq", [CG, 1], F32, isOutput=False)
    bk = nc.declare_dram_parameter("bk", [CG, 1], F32, isOutput=False)
    ones = nc.declare_dram_parameter("ones", [1, DK], F16, isOutput=False)
    outT = nc.declare_dram_parameter("outT", [D, S], F32, isOutput=True)

    KT = D // 128        # 8 contraction tiles for projections
    SC = S // 512        # 4 s-chunks of 512
    ST = S // 128        # 16 s-tiles of 128
    MT = CG // 128       # 2 dk-tiles (head pairs)

    with tile.TileContext(nc) as tc, ExitStack() as ctx:
        wt = ctx.enter_context(tc.tile_pool(name="wt", bufs=1))
        xb = ctx.enter_context(tc.tile_pool(name="xb", bufs=64))
        qk = ctx.enter_context(tc.tile_pool(name="qk", bufs=1))
        va = ctx.enter_context(tc.tile_pool(name="va", bufs=1))
        pt = ctx.enter_context(tc.tile_pool(name="pt", bufs=4))
        oc = ctx.enter_context(tc.tile_pool(name="oc", bufs=1))
        nt = ctx.enter_context(tc.tile_pool(name="nt", bufs=8))
        rc = ctx.enter_context(tc.tile_pool(name="rc", bufs=4))
        so = ctx.enter_context(tc.tile_pool(name="so", bufs=4))
        pp = ctx.enter_context(tc.tile_pool(name="pp", bufs=2, space="PSUM"))

        _dma_i = [0]

        def dma(out, in_):
            eng = nc.sync if (_dma_i[0] % 2 == 0) else nc.scalar
            _dma_i[0] += 1
            eng.dma_start(out, in_)

        # ---- weights + biases first (small, unblock all matmuls)
        wk_sb = wt.tile([128, KT * CG], BF16, tag="wk", name="wk")
        wv_sb = wt.tile([128, KT * CG], BF16, tag="wv", name="wv")
        wq_sb = wt.tile([128, KT * CG], BF16, tag="wq", name="wq")
        bq_sb = [wt.tile([128, 1], F32, tag=f"bq{m}", name=f"bq{m}") for m in range(MT)]
        bk_sb = [wt.tile([128, 1], F32, tag=f"bk{m}", name=f"bk{m}") for m in range(MT)]
        ones_sb = wt.tile([1, DK], F16, tag="ones")
        dma(wk_sb[:].rearrange("p (k c) -> p k c", k=KT),
            wkT[:].rearrange("(k p) c -> p k c", p=128))
        for m in range(MT):
            dma(bk_sb[m][:], bk[m * 128:(m + 1) * 128, :])
            dma(bq_sb[m][:], bq[m * 128:(m + 1) * 128, :])
        nc.sync.dma_start(ones_sb[:], ones[:])

        def wslice(w, kt, lo, hi):
            return w[:, kt * CG + lo:kt * CG + hi]

        # ---- x loads: one 3D-strided DMA per 512-column chunk -> [128, 4096]
        # tiles (kt-major in the free dim). Priority: K, Q chunk 0, V, Q rest.
        def chunk(dram, sc):
            p = xb.tile([128, KT * 512], BF16, tag="xch", name="xch", bufs=12)
            dma(p[:].rearrange("p (k c) -> p k c", k=KT),
                dram[:, sc * 512:(sc + 1) * 512]
                .rearrange("(k p) c -> p k c", p=128))
            return p

        xk_ch, xq_ch, xv_ch = {}, {}, {}
        for sc in range(SC):
            xk_ch[sc] = chunk(xkT, sc)
        xq_ch[0] = chunk(xqT, 0)
        dma(wq_sb[:].rearrange("p (k c) -> p k c", k=KT),
            wqT[:].rearrange("(k p) c -> p k c", p=128))
        dma(wv_sb[:].rearrange("p (k c) -> p k c", k=KT),
            wvT[:].rearrange("(k p) c -> p k c", p=128))
        for g in range(SC):
            xv_ch[g] = chunk(xvT, g)
        for sc in range(1, SC):
            xq_ch[sc] = chunk(xqT, sc)

        kh = [qk.tile([128, S], BF16, tag=f"kh{m}", name=f"kh{m}") for m in range(MT)]
        qh = [qk.tile([128, S], BF16, tag=f"qh{m}", name=f"qh{m}") for m in range(MT)]
        va_t = [va.tile([128, HG * (DK + 1)], BF16, tag=f"va{st}", name=f"va{st}")
                for st in range(ST)]
        ocat = [oc.tile([128, S], BF16, tag=f"oc{c}", name=f"oc{c}") for c in range(MT)]
        wo_sb = [wt.tile([128, D], BF16, tag=f"wo{c}", name=f"wo{c}")
                 for c in range(MT)]
        for c in range(MT):
            dma(wo_sb[c][:], woT[c * 128:(c + 1) * 128, :])

        # ---- K projection (chases the xk pieces sc by sc)
        for sc in range(SC):
            for m in range(MT):
                pk = pp.tile([128, 512], F32, tag="st", name="pk")
                for kt in range(KT):
                    nc.tensor.matmul(pk[:], wslice(wk_sb, kt, m * 128, (m + 1) * 128),
                                     xk_ch[sc][:, kt * 512:(kt + 1) * 512],
                                     start=(kt == 0), stop=(kt == KT - 1))
                nc.vector.tensor_scalar_add(
                    kh[m][:, sc * 512:(sc + 1) * 512], pk[:], bk_sb[m][:])

        def qproj_half(sc, m):
            pq = pp.tile([128, 512], F32, tag="st", name="pq")
            for kt in range(KT):
                nc.tensor.matmul(pq[:], wslice(wq_sb, kt, m * 128, (m + 1) * 128),
                                 xq_ch[sc][:, kt * 512:(kt + 1) * 512],
                                 start=(kt == 0), stop=(kt == KT - 1))
            nc.vector.tensor_scalar_add(
                qh[m][:, sc * 512:(sc + 1) * 512], pq[:], bq_sb[m][:])

        def va_make(st):
            pv = pp.tile([128, CG], F32, tag="st", name="pv")
            g, j = st // 4, st % 4
            for kt in range(KT):
                nc.tensor.matmul(pv[:],
                                 xv_ch[g][:, kt * 512 + j * 128:kt * 512 + (j + 1) * 128],
                                 wslice(wv_sb, kt, 0, CG),
                                 start=(kt == 0), stop=(kt == KT - 1))
            vt = va_t[st]
            nc.vector.tensor_copy(
                vt[:].rearrange("p (h w) -> p h w", h=HG)[:, :, 0:DK],
                pv[:].rearrange("p (h w) -> p h w", h=HG),
            )
            nc.gpsimd.memset(
                vt[:].rearrange("p (h w) -> p h w", h=HG)[:, :, DK:DK + 1], 1.0)

        def norm_one(item, i):
            hp_, qg_, tmps, rpair = item
            bc = pp.tile([DK, 512], F32, tag="bc", bufs=1, name="bc")
            nc.tensor.matmul(bc[:], ones_sb[:], rpair[i][:], start=True, stop=True)
            nc.vector.tensor_mul(
                ocat[hp_][i * DK:(i + 1) * DK, qg_ * 512:(qg_ + 1) * 512],
                tmps[i][:], bc[:])

        def wo_chunk(wg, mts):
            for mt in mts:
                po = pp.tile([128, 512], F32, tag="st", name="po")
                for ct in range(MT):
                    nc.tensor.matmul(po[:], wo_sb[ct][:, mt * 128:(mt + 1) * 128],
                                     ocat[ct][:, wg * 512:(wg + 1) * 512],
                                     start=(ct == 0), stop=(ct == MT - 1))
                stg = so.tile([128, 512], F32, tag="so")
                nc.vector.tensor_copy(stg[:], po[:])
                nc.sync.dma_start(
                    outT[mt * 128:(mt + 1) * 128, wg * 512:(wg + 1) * 512], stg[:])

        qproj_half(0, 0)
        qproj_half(0, 1)
        for st in range(ST):
            va_make(st)

        pending = []   # (hp, qg, tmps, rpair), flushed one qg later
        for qg in range(SC):
            prev = [p for p in pending if p[1] == qg - 1]
            for hp in range(MT):
                weave = {}

                def add(kt, fn):
                    weave.setdefault(kt, []).append(fn)

                if hp == 0 and prev:
                    add(3, lambda: norm_one(prev[0], 0))
                    add(5, lambda: norm_one(prev[0], 1))
                    add(9, lambda: norm_one(prev[1], 0))
                    add(11, lambda: norm_one(prev[1], 1))
                    add(14, lambda: wo_chunk(qg - 1, range(0, 4)))
                if hp == 1 and qg + 1 < SC:
                    add(6, lambda: qproj_half(qg + 1, 0))
                    add(12, lambda: qproj_half(qg + 1, 1))
                if hp == 1 and prev:
                    add(2, lambda: wo_chunk(qg - 1, range(4, 8)))

                ots = [pp.tile([DK + 1, 512], F32, tag="ot", bufs=3, name="ot")
                       for _ in range(2)]
                for kt in range(ST):
                    stp = pp.tile([128, 1024], F32, tag="st")
                    for i in range(2):
                        nc.tensor.matmul(
                            stp[:, i * 512:(i + 1) * 512],
                            kh[hp][i * 64:(i + 1) * 64, kt * 128:(kt + 1) * 128],
                            qh[hp][i * 64:(i + 1) * 64, qg * 512:(qg + 1) * 512],
                            start=True, stop=True)
                    ptp = pt.tile([128, 1024], BF16, tag="pt")
                    nc.scalar.activation(ptp[:], stp[:], Exp, scale=0.125)
                    for i in range(2):
                        h = 2 * hp + i
                        nc.tensor.matmul(
                            ots[i][:],
                            va_t[kt][:, h * (DK + 1):(h + 1) * (DK + 1)],
                            ptp[:, i * 512:(i + 1) * 512],
                            start=(kt == 0), stop=(kt == ST - 1))
                    for fn in weave.get(kt, []):
                        fn()
                tmps, rpair = [], []
                for i in range(2):
                    tmp = nt.tile([DK, 512], F32, tag="nt")
                    nc.vector.tensor_copy(tmp[:], ots[i][0:DK, :])
                    tmps.append(tmp)
                dens = []
                for i in range(2):
                    den = rc.tile([1, 512], F32, tag="den", bufs=4, name="den")
                    nc.vector.tensor_copy(den[:], ots[i][DK:DK + 1, :])
                    dens.append(den)
                for i in range(2):
                    raf = rc.tile([1, 512], F32, tag="raf", bufs=4, name="raf")
                    nc.vector.reciprocal_approx_fast(raf[:], dens[i][:])
                    rec = rc.tile([1, 512], F16, tag="rc", bufs=8, name="rec")
                    with nc.allow_low_precision(reason="fp16 softmax denom recip"):
                        nc.vector.tensor_copy(rec[:], raf[:])
                    rpair.append(rec)
                pending.append((hp, qg, tmps, rpair))
            pending = [p for p in pending if p[1] >= qg]
        # tail: last q-group's normalize + output projection
        last = [p for p in pending if p[1] == SC - 1]
        for item in last:
            norm_one(item, 0)
            norm_one(item, 1)
        wo_chunk(SC - 1, range(0, 8))

    nc.compile()
    return nc


def _get_nc():
    global _nc_cache
    if _nc_cache is None:
        _nc_cache = _build()
    return _nc_cache


def kernel(q, k, v, mask, Wq, bq, Wk, bk, Wv, bv, Wo, bo):
    import ml_dtypes
    from concourse.bass_utils import run_bass_kernel_spmd

    Bb = ml_dtypes.bfloat16
    q = np.asarray(q, np.float32)
    k = np.asarray(k, np.float32)
    v = np.asarray(v, np.float32)
    Wq = np.asarray(Wq, np.float32)
    Wk = np.asarray(Wk, np.float32)
    Wv = np.asarray(Wv, np.float32)
    Wo = np.asarray(Wo, np.float32)
    bq = np.asarray(bq, np.float32)
    bk = np.asarray(bk, np.float32)
    bv = np.asarray(bv, np.float32)
    bo = np.asarray(bo, np.float32)

    nc = _get_nc()
    ones = np.ones((1, DK), np.float16)
    xT = {}
    for b in range(B):
        xT[b] = (np.ascontiguousarray(q[b].T).astype(Bb),
                 np.ascontiguousarray(k[b].T).astype(Bb),
                 np.ascontiguousarray(v[b].T).astype(Bb))
    in_maps = []
    for c in range(N_CORES):
        b, g = c // G, c % G
        cols = slice(g * CG, (g + 1) * CG)
        in_maps.append({
            "xqT": xT[b][0], "xkT": xT[b][1], "xvT": xT[b][2],
            "wqT": np.ascontiguousarray(Wq[cols, :].T).astype(Bb),
            "wkT": np.ascontiguousarray(Wk[cols, :].T).astype(Bb),
            "wvT": np.ascontiguousarray(Wv[cols, :].T).astype(Bb),
            "woT": np.ascontiguousarray(Wo[:, cols].T).astype(Bb),
            "bq": np.ascontiguousarray(bq[cols]).reshape(CG, 1),
            "bk": np.ascontiguousarray(bk[cols]).reshape(CG, 1),
            "ones": ones,
        })

    global _last_in_maps
    _last_in_maps = in_maps
    res = run_bass_kernel_spmd(nc, in_maps, list(range(N_CORES)))

    out = np.zeros((B, S, D), np.float32)
    for c in range(N_CORES):
        b = c // G
        out[b] += res.results[c]["outT"].T
    # bv/bo contribution: softmax rows sum to 1, so V-bias passes through as
    # a constant row; fold both host-side.
    out += (bv @ Wo.T + bo)[None, None, :]
    return out


# revision 20
# speedup vs baseline: 1.1614x; 1.1614x over previous
"""Multi-head attention Trainium2 kernel (B=2, S=2048, D=1024, H=16).

Sharding: 8 cores = 2 batches x 4 head-groups (4 heads each).
Each core computes its head-group's attention for its batch plus the
partial output projection (Wo columns for its heads); the host sums the
4 partials per batch.

Device layout is feature-on-partition throughout (inputs pre-transposed
and pre-cast to bf16 on host), zero on-device transposes:
  QhT/KhT = W @ x^T            [dk, s]
  S^T     = KhT^T-slices @ QhT [s_k, s_q]   (K=64, head pairs row-packed)
  P^T     = exp(S^T / 8)                    (ScalarE, fused scale)
  O^T     = V_aug^T-slices @ P^T [dk+1, s_q] (ones col => row 64 = denom)
  out^T   = Wo_g^T @ OcatT^norm  [e, s]
Softmax normalization: fp16 reciprocal of the denom row + K=1
ones-matmul broadcast across partitions + DVE multiply.
bf16 matmul operands, fp32 PSUM. bq/bk applied on device per-partition;
bv/bo folded in on host (softmax rows sum to 1 => V-bias is additive).
The boolean mask input is all-ones per the problem spec and is ignored.
"""
import sys

sys.path.insert(0, "/opt/trn_rl_repo")

import numpy as np
from contextlib import ExitStack

B, S, D, H = 2, 2048, 1024, 16
DK = D // H          # 64
G = 4                # head-groups (cores per batch)
HG = H // G          # 4 heads per group
CG = HG * DK         # 256 features per group
N_CORES = 8

_nc_cache = None
_last_in_maps = None


def _build():
    import concourse.tile as tile
    from concourse import bacc, mybir

    F32 = mybir.dt.float32
    F16 = mybir.dt.float16
    BF16 = mybir.dt.bfloat16
    Exp = mybir.ActivationFunctionType.Exp

    nc = bacc.Bacc("TRN2", target_bir_lowering=False, debug=False)

    xqT = nc.declare_dram_parameter("xqT", [D, S], BF16, isOutput=False)
    xkT = nc.declare_dram_parameter("xkT", [D, S], BF16, isOutput=False)
    xvT = nc.declare_dram_parameter("xvT", [D, S], BF16, isOutput=False)
    wqT = nc.declare_dram_parameter("wqT", [D, CG], BF16, isOutput=False)
    wkT = nc.declare_dram_parameter("wkT", [D, CG], BF16, isOutput=False)
    wvT = nc.declare_dram_parameter("wvT", [D, CG], BF16, isOutput=False)
    woT = nc.declare_dram_parameter("woT", [CG, D], BF16, isOutput=False)
    bq = nc.declare_dram_parameter("bq", [CG, 1], F32, isOutput=False)
    bk = nc.declare_dram_parameter("bk", [CG, 1], F32, isOutput=False)
    ones = nc.declare_dram_parameter("ones", [1, DK], F16, isOutput=False)
    outT = nc.declare_dram_parameter("outT", [D, S], F32, isOutput=True)

    KT = D // 128        # 8 contraction tiles for projections
    SC = S // 512        # 4 s-chunks of 512
    ST = S // 128        # 16 s-tiles of 128
    MT = CG // 128       # 2 dk-tiles (head pairs)

    with tile.TileContext(nc) as tc, ExitStack() as ctx:
        wt = ctx.enter_context(tc.tile_pool(name="wt", bufs=1))
        xb = ctx.enter_context(tc.tile_pool(name="xb", bufs=64))
        qk = ctx.enter_context(tc.tile_pool(name="qk", bufs=1))
        va = ctx.enter_context(tc.tile_pool(name="va", bufs=1))
        pt = ctx.enter_context(tc.tile_pool(name="pt", bufs=4))
        oc = ctx.enter_context(tc.tile_pool(name="oc", bufs=1))
        nt = ctx.enter_context(tc.tile_pool(name="nt", bufs=8))
        rc = ctx.enter_context(tc.tile_pool(name="rc", bufs=4))
        so = ctx.enter_context(tc.tile_pool(name="so", bufs=4))
        pp = ctx.enter_context(tc.tile_pool(name="pp", bufs=2, space="PSUM"))

        _dma_i = [0]

        def dma(out, in_):
            eng = nc.sync if (_dma_i[0] % 2 == 0) else nc.scalar
            _dma_i[0] += 1
            eng.dma_start(out, in_)

        # ---- weights + biases first (small, unblock all matmuls)
        wk_sb = wt.tile([128, KT * CG], BF16, tag="wk", name="wk")
        wv_sb = wt.tile([128, KT * CG], BF16, tag="wv", name="wv")
        wq_sb = wt.tile([128, KT * CG], BF16, tag="wq", name="wq")
        bq_sb = [wt.tile([128, 1], F32, tag=f"bq{m}", name=f"bq{m}") for m in range(MT)]
        bk_sb = [wt.tile([128, 1], F32, tag=f"bk{m}", name=f"bk{m}") for m in range(MT)]
        ones_sb = wt.tile([1, DK], F16, tag="ones")
        dma(wk_sb[:].rearrange("p (k c) -> p k c", k=KT),
            wkT[:].rearrange("(k p) c -> p k c", p=128))
        dma(wv_sb[:].rearrange("p (k c) -> p k c", k=KT),
            wvT[:].rearrange("(k p) c -> p k c", p=128))
        dma(wq_sb[:].rearrange("p (k c) -> p k c", k=KT),
            wqT[:].rearrange("(k p) c -> p k c", p=128))
        for m in range(MT):
            dma(bk_sb[m][:], bk[m * 128:(m + 1) * 128, :])
            dma(bq_sb[m][:], bq[m * 128:(m + 1) * 128, :])
        nc.sync.dma_start(ones_sb[:], ones[:])

        def wslice(w, kt, lo, hi):
            return w[:, kt * CG + lo:kt * CG + hi]

        # ---- x loads: one 3D-strided DMA per 512-column chunk -> [128, 4096]
        # tiles (kt-major in the free dim). Priority: K, Q chunk 0, V, Q rest.
        def chunk(dram, sc):
            p = xb.tile([128, KT * 512], BF16, tag="xch", name="xch", bufs=12)
            dma(p[:].rearrange("p (k c) -> p k c", k=KT),
                dram[:, sc * 512:(sc + 1) * 512]
                .rearrange("(k p) c -> p k c", p=128))
            return p

        xk_ch, xq_ch, xv_ch = {}, {}, {}
        for sc in range(SC):
            xk_ch[sc] = chunk(xkT, sc)
        xq_ch[0] = chunk(xqT, 0)
        for g in range(SC):
            xv_ch[g] = chunk(xvT, g)
        for sc in range(1, SC):
            xq_ch[sc] = chunk(xqT, sc)

        kh = [qk.tile([128, S], BF16, tag=f"kh{m}", name=f"kh{m}") for m in range(MT)]
        qh = [qk.tile([128, S], BF16, tag=f"qh{m}", name=f"qh{m}") for m in range(MT)]
        va_t = [va.tile([128, HG * (DK + 1)], BF16, tag=f"va{st}", name=f"va{st}")
                for st in range(ST)]
        ocat = [oc.tile([128, S], BF16, tag=f"oc{c}", name=f"oc{c}") for c in range(MT)]
        wo_sb = [wt.tile([128, D], BF16, tag=f"wo{c}", name=f"wo{c}")
                 for c in range(MT)]
        for c in range(MT):
            dma(wo_sb[c][:], woT[c * 128:(c + 1) * 128, :])

        # ---- K projection (chases the xk pieces sc by sc)
        for sc in range(SC):
            for m in range(MT):
                pk = pp.tile([128, 512], F32, tag="st", name="pk")
                for kt in range(KT):
                    nc.tensor.matmul(pk[:], wslice(wk_sb, kt, m * 128, (m + 1) * 128),
                                     xk_ch[sc][:, kt * 512:(kt + 1) * 512],
                                     start=(kt == 0), stop=(kt == KT - 1))
                nc.vector.tensor_scalar_add(
                    kh[m][:, sc * 512:(sc + 1) * 512], pk[:], bk_sb[m][:])

        def qproj_half(sc, m):
            pq = pp.tile([128, 512], F32, tag="st", name="pq")
            for kt in range(KT):
                nc.tensor.matmul(pq[:], wslice(wq_sb, kt, m * 128, (m + 1) * 128),
                                 xq_ch[sc][:, kt * 512:(kt + 1) * 512],
                                 start=(kt == 0), stop=(kt == KT - 1))
            nc.vector.tensor_scalar_add(
                qh[m][:, sc * 512:(sc + 1) * 512], pq[:], bq_sb[m][:])

        def va_make(st):
            pv = pp.tile([128, CG], F32, tag="st", name="pv")
            g, j = st // 4, st % 4
            for kt in range(KT):
                nc.tensor.matmul(pv[:],
                                 xv_ch[g][:, kt * 512 + j * 128:kt * 512 + (j + 1) * 128],
                                 wslice(wv_sb, kt, 0, CG),
                                 start=(kt == 0), stop=(kt == KT - 1))
            vt = va_t[st]
            nc.vector.tensor_copy(
                vt[:].rearrange("p (h w) -> p h w", h=HG)[:, :, 0:DK],
                pv[:].rearrange("p (h w) -> p h w", h=HG),
            )
            nc.gpsimd.memset(
                vt[:].rearrange("p (h w) -> p h w", h=HG)[:, :, DK:DK + 1], 1.0)

        def norm_one(item, i):
            hp_, qg_, tmps, rpair = item
            bc = pp.tile([DK, 512], F32, tag="bc", bufs=1, name="bc")
            nc.tensor.matmul(bc[:], ones_sb[:], rpair[i][:], start=True, stop=True)
            nc.vector.tensor_mul(
                ocat[hp_][i * DK:(i + 1) * DK, qg_ * 512:(qg_ + 1) * 512],
                tmps[i][:], bc[:])

        def wo_chunk(wg, mts):
            for mt in mts:
                po = pp.tile([128, 512], F32, tag="st", name="po")
                for ct in range(MT):
                    nc.tensor.matmul(po[:], wo_sb[ct][:, mt * 128:(mt + 1) * 128],
                                     ocat[ct][:, wg * 512:(wg + 1) * 512],
                                     start=(ct == 0), stop=(ct == MT - 1))
                stg = so.tile([128, 512], F32, tag="so")
                nc.vector.tensor_copy(stg[:], po[:])
                nc.sync.dma_start(
                    outT[mt * 128:(mt + 1) * 128, wg * 512:(wg + 1) * 512], stg[:])

        qproj_half(0, 0)
        qproj_half(0, 1)
        for st in range(4):
            va_make(st)

        pending = []   # (hp, qg, tmps, rpair), flushed one qg later
        for qg in range(SC):
            prev = [p for p in pending if p[1] == qg - 1]
            for hp in range(MT):
                weave = {}

                def add(kt, fn):
                    weave.setdefault(kt, []).append(fn)

                if qg == 0 and hp == 0:
                    for st in range(4, ST):
                        add(st - 4, lambda st=st: va_make(st))
                if hp == 0 and prev:
                    add(3, lambda: norm_one(prev[0], 0))
                    add(5, lambda: norm_one(prev[0], 1))
                    add(9, lambda: norm_one(prev[1], 0))
                    add(11, lambda: norm_one(prev[1], 1))
                    add(14, lambda: wo_chunk(qg - 1, range(0, 4)))
                if hp == 1 and qg + 1 < SC:
                    add(6, lambda: qproj_half(qg + 1, 0))
                    add(12, lambda: qproj_half(qg + 1, 1))
                if hp == 1 and prev:
                    add(2, lambda: wo_chunk(qg - 1, range(4, 8)))

                ots = [pp.tile([DK + 1, 512], F32, tag="ot", bufs=3, name="ot")
                       for _ in range(2)]
                for kt in range(ST):
                    stp = pp.tile([128, 1024], F32, tag="st")
                    for i in range(2):
                        nc.tensor.matmul(
                            stp[:, i * 512:(i + 1) * 512],
                            kh[hp][i * 64:(i + 1) * 64, kt * 128:(kt + 1) * 128],
                            qh[hp][i * 64:(i + 1) * 64, qg * 512:(qg + 1) * 512],
                            start=True, stop=True)
                    ptp = pt.tile([128, 1024], BF16, tag="pt")
                    nc.scalar.activation(ptp[:], stp[:], Exp, scale=0.125)
                    for i in range(2):
                        h = 2 * hp + i
                        nc.tensor.matmul(
                            ots[i][:],
                            va_t[kt][:, h * (DK + 1):(h + 1) * (DK + 1)],
                            ptp[:, i * 512:(i + 1) * 512],
                            start=(kt == 0), stop=(kt == ST - 1))
                    for fn in weave.get(kt, []):
                        fn()
                tmps, rpair = [], []
                for i in range(2):
                    tmp = nt.tile([DK, 512], F32, tag="nt")
                    nc.vector.tensor_copy(tmp[:], ots[i][0:DK, :])
                    tmps.append(tmp)
                dens = []
                for i in range(2):
                    den = rc.tile([1, 512], F32, tag="den", bufs=4, name="den")
                    nc.vector.tensor_copy(den[:], ots[i][DK:DK + 1, :])
                    dens.append(den)
                for i in range(2):
                    raf = rc.tile([1, 512], F32, tag="raf", bufs=4, name="raf")
                    nc.vector.reciprocal_approx_fast(raf[:], dens[i][:])
                    rec = rc.tile([1, 512], F16, tag="rc", bufs=8, name="rec")
                    with nc.allow_low_precision(reason="fp16 softmax denom recip"):
                        nc.vector.tensor_copy(rec[:], raf[:])
                    rpair.append(rec)
                pending.append((hp, qg, tmps, rpair))
            pending = [p for p in pending if p[1] >= qg]
        # tail: last q-group's normalize + output projection
        last = [p for p in pending if p[1] == SC - 1]
        for item in last:
            norm_one(item, 0)
            norm_one(item, 1)
        wo_chunk(SC - 1, range(0, 8))

    nc.compile()
    return nc


def _get_nc():
    global _nc_cache
    if _nc_cache is None:
        _nc_cache = _build()
    return _nc_cache


def kernel(q, k, v, mask, Wq, bq, Wk, bk, Wv, bv, Wo, bo):
    import ml_dtypes
    from concourse.bass_utils import run_bass_kernel_spmd

    Bb = ml_dtypes.bfloat16
    q = np.asarray(q, np.float32)
    k = np.asarray(k, np.float32)
    v = np.asarray(v, np.float32)
    Wq = np.asarray(Wq, np.float32)
    Wk = np.asarray(Wk, np.float32)
    Wv = np.asarray(Wv, np.float32)
    Wo = np.asarray(Wo, np.float32)
    bq = np.asarray(bq, np.float32)
    bk = np.asarray(bk, np.float32)
    bv = np.asarray(bv, np.float32)
    bo = np.asarray(bo, np.float32)

    nc = _get_nc()
    ones = np.ones((1, DK), np.float16)
    xT = {}
    for b in range(B):
        xT[b] = (np.ascontiguousarray(q[b].T).astype(Bb),
                 np.ascontiguousarray(k[b].T).astype(Bb),
                 np.ascontiguousarray(v[b].T).astype(Bb))
    in_maps = []
    for c in range(N_CORES):
        b, g = c // G, c % G
        cols = slice(g * CG, (g + 1) * CG)
        in_maps.append({
            "xqT": xT[b][0], "xkT": xT[b][1], "xvT": xT[b][2],
            "wqT": np.ascontiguousarray(Wq[cols, :].T).astype(Bb),
            "wkT": np.ascontiguousarray(Wk[cols, :].T).astype(Bb),
            "wvT": np.ascontiguousarray(Wv[cols, :].T).astype(Bb),
            "woT": np.ascontiguousarray(Wo[:, cols].T).astype(Bb),
            "bq": np.ascontiguousarray(bq[cols]).reshape(CG, 1),
            "bk": np.ascontiguousarray(bk[cols]).reshape(CG, 1),
            "ones": ones,
        })

    global _last_in_maps
    _last_in_maps = in_maps
    res = run_bass_kernel_spmd(nc, in_maps, list(range(N_CORES)))

    out = np.zeros((B, S, D), np.float32)
    for c in range(N_CORES):
        b = c // G
        out[b] += res.results[c]["outT"].T
    # bv/bo contribution: softmax rows sum to 1, so V-bias passes through as
    # a constant row; fold both host-side.
    out += (bv @ Wo.T + bo)[None, None, :]
    return out
